# revision 4
# baseline (speedup 1.0000x reference)
"""BertBiLSTMCRF loss kernel for 8 Trainium2 NeuronCores.

Sharding: data-parallel over batch (B=32 -> 4 sentences/core). The BERT
encoder (>95% of FLOPs) runs on-device in raw Bass. Activations are kept
in transposed layout hT=[H, tokens] on chip so every GEMM consumes
weights in their stored [in,out] layout as lhsT with no activation
transposes; attention computes S^T (k on partitions), uses unnormalized
exp (scores are tiny after LN + 0.02-scale weights) and gets the softmax
denominator via a ones-column matmul, so no partition-dim max/sum is
ever needed. The BiLSTM/CRF tail (small FLOPs, serial scans) runs on
host, as does the embedding gather.
"""
import os
import numpy as np
from scipy.special import erf

V, H, NL, NH, S, B, HL, T = 30522, 768, 12, 12, 256, 32, 256, 9
DH = H // NH
FF = 4 * H
NCORES = 8
BL = B // NCORES          # sentences per core
TOK = BL * S              # tokens per core (1024)
KT = H // 128             # 6 k-tiles over hidden
MT_TOK = TOK // 128       # 8 token m-tiles
FP32R = os.environ.get("KERNEL_NO_FP32R", "") == ""
USE_DEVICE = os.environ.get("KERNEL_HOST", "") == ""
DEV_LAYERS = int(os.environ.get("KERNEL_LAYERS", str(NL)))

LAST_HW_NS = None
_CACHE = {}


# ---------------------------------------------------------------- host math
def _ln_np(x, g, b):
    m = x.mean(-1, keepdims=True)
    v = ((x - m) ** 2).mean(-1, keepdims=True)
    return (x - m) / np.sqrt(v + 1e-12) * g + b


def _gelu_np(x):
    return (0.5 * x * (1.0 + erf(x / np.float32(np.sqrt(2.0))))).astype(np.float32)


def _sigmoid_np(x):
    return 1.0 / (1.0 + np.exp(-x))


def _bert_host(h, a, n_layers=NL):
    Bc = h.shape[0]
    for l in range(n_layers):
        qkv = h @ a['Wqkv'][l] + a['bqkv'][l]
        q, k, v = [t.reshape(Bc, S, NH, DH) for t in np.split(qkv, 3, axis=-1)]
        sc = np.einsum('bqhd,bkhd->bhqk', q, k) / np.float32(np.sqrt(DH))
        sc = sc - sc.max(-1, keepdims=True)
        p = np.exp(sc)
        p = p / p.sum(-1, keepdims=True)
        ctx = np.einsum('bhqk,bkhd->bqhd', p, v).reshape(Bc, S, H)
        h = _ln_np(h + ctx @ a['Wo'][l] + a['bo'][l], a['ln1_g'][l], a['ln1_b'][l])
        ff = _gelu_np(h @ a['W1'][l] + a['b1'][l]) @ a['W2'][l] + a['b2'][l]
        h = _ln_np(h + ff, a['ln2_g'][l], a['ln2_b'][l])
    return h


def _lstm_host(x, Wih, Whh, bih, bhh, reverse):
    Bc = x.shape[0]
    pre = np.swapaxes(x, 0, 1) @ Wih.T + (bih + bhh)  # [S,B,4H]
    hs = np.zeros((S, Bc, HL), np.float32)
    h = np.zeros((Bc, HL), np.float32)
    c = np.zeros((Bc, HL), np.float32)
    order = range(S - 1, -1, -1) if reverse else range(S)
    for t in order:
        g = pre[t] + h @ Whh.T
        i, f, gg, o = np.split(g, 4, axis=-1)
        c = _sigmoid_np(f) * c + _sigmoid_np(i) * np.tanh(gg)
        h = _sigmoid_np(o) * np.tanh(c)
        hs[t] = h
    return np.swapaxes(hs, 0, 1)


def _logsumexp(a, axis):
    m = a.max(axis=axis, keepdims=True)
    return (np.log(np.exp(a - m).sum(axis=axis, keepdims=True)) + m).squeeze(axis)


def _crf_host(logits, labels, maskf, crf_start, crf_end, crf_trans):
    em = np.take_along_axis(logits, labels[..., None], -1)[..., 0]
    tr = crf_trans[labels[:, :-1], labels[:, 1:]]
    last_idx = maskf.sum(1).astype(np.int32) - 1
    last_tag = np.take_along_axis(labels, last_idx[:, None], 1)[:, 0]
    num = (crf_start[labels[:, 0]] + em[:, 0]
           + ((em[:, 1:] + tr) * maskf[:, 1:]).sum(1) + crf_end[last_tag])
    alpha = crf_start + logits[:, 0]
    for t in range(1, S):
        nxt = _logsumexp(alpha[:, :, None] + crf_trans[None] + logits[:, t][:, None, :], 1)
        alpha = np.where(maskf[:, t][:, None] > 0, nxt, alpha)
    den = _logsumexp(alpha + crf_end, -1)
    return den - num


# ------------------------------------------------------------ device program
class Prog:
    """Raw-Bass multi-engine program recorder with conservative sync:
    each op waits until everything its producer engines emitted so far is
    done. Duplicate waits are elided per consumer engine. The DMA
    semaphore rotates per layer to stay far from counter limits."""

    def __init__(self):
        self.ops = {e: [] for e in ("pe", "act", "dve", "dma")}
        self.counts = {}              # sem name -> emitted count
        self.seen = {e: {} for e in self.ops}
        self.cur_dma = "dmaS0"
        self.sem_names = {"pe", "act", "dve", "dmaS0"}

    def next_dma_sem(self, name):
        self.cur_dma = name
        self.sem_names.add(name)

    def _resolve(self, dep):
        if dep == "dma":
            return [s for s in self.sem_names if s.startswith("dmaS")]
        return [dep]

    def emit(self, engine, fn, deps=()):
        waits = []
        for d in deps:
            for sem in self._resolve(d):
                if sem == engine:
                    continue
                val = self.counts.get(sem, 0)
                if val > 0 and self.seen[engine].get(sem, -1) < val:
                    waits.append((sem, val))
                    self.seen[engine][sem] = val
        sem_self = self.cur_dma if engine == "dma" else engine
        inc = 16 if engine == "dma" else 1
        self.counts[sem_self] = self.counts.get(sem_self, 0) + inc
        self.ops[engine].append((waits, fn, sem_self, inc))

    def replay(self, engine, eng, sems):
        for waits, fn, sem_self, inc in self.ops[engine]:
            for name, val in waits:
                eng.wait_ge(sems[name], val)
            fn().then_inc(sems[sem_self], inc)


def _build_encoder(n_layers):
    import concourse.bass as bass
    import concourse.mybir as mybir
    from contextlib import ExitStack
    dt = mybir.dt
    f32 = dt.float32
    AF = mybir.ActivationFunctionType
    ALU = mybir.AluOpType

    nc = bass.Bass()
    ctx = ExitStack()

    def mmdt(ap):
        return ap.bitcast(dt.float32r) if FP32R else ap

    def R(ap):
        # round-on-write for tiles later consumed by fp32r matmuls
        return ap.bitcast(dt.float32r) if FP32R else ap

    # ---- DRAM parameters
    hT0 = nc.declare_dram_parameter("hT0", [H, TOK], f32, isOutput=False)
    Wqkv = nc.declare_dram_parameter("Wqkv", [NL, H, 3 * H], f32, isOutput=False)
    Wo = nc.declare_dram_parameter("Wo", [NL, H, H], f32, isOutput=False)
    W1 = nc.declare_dram_parameter("W1", [NL, H, FF], f32, isOutput=False)
    W2 = nc.declare_dram_parameter("W2", [NL, FF, H], f32, isOutput=False)
    biasall = nc.declare_dram_parameter("biasall", [NL, 128, 80], f32, isOutput=False)
    consts = nc.declare_dram_parameter("consts", [128, 1024], f32, isOutput=False)
    onesd = nc.declare_dram_parameter("onesd", [128, 1], f32, isOutput=False)
    hTout = nc.declare_dram_parameter("hTout", [H, TOK], f32, isOutput=True)
    zscr = nc.dram_tensor("zscr", [4, 3072], f32)

    # ---- on-chip tensors
    sbt = lambda nm, shape: ctx.enter_context(nc.sbuf_tensor(nm, shape, f32))
    hT = sbt("hT", [128, KT, TOK])
    h1T = sbt("h1T", [128, KT, TOK])
    ctxT = sbt("ctxT", [128, KT * TOK])   # flat; viewed [128, 6, 1024]
    big = sbt("bigb", [128, 12, TOK])     # qkT in attn; ff1 tiles 0-11; LN sq
    vbuf = sbt("vbuf", [128, KT * TOK])   # flat; v=[128,8,768] / ff1 18-23
    wsl = sbt("wsl", [128, 2, 3072])      # weight slab, 2 slots
    bias = sbt("biassb", [128, 80])
    csts = sbt("csts", [128, 904])
    stats = sbt("stats", [1, 2048])   # col blocks: mean | E2/var/istd
    stats2 = sbt("stats2", [1, 1024])  # istd
    zbuf4 = sbt("zbuf4", [97, 3072])   # Z at partition bases 0/32/64/96
    zbuf = sbt("zbuf", [12, 1024])     # Z reshaped for broadcast matmul
    expS = sbt("expS", [128, 2, S])
    onesr = sbt("onesr", [128, 1])

    psA = ctx.enter_context(nc.psum_tensor("psA", [128, 1024], f32))
    psB = ctx.enter_context(nc.psum_tensor("psB", [128, 1024], f32))
    psS = ctx.enter_context(nc.psum_tensor("psS", [128, 2, S], f32))
    psC = ctx.enter_context(nc.psum_tensor("psC", [128, S], f32))

    ctxTv = ctxT[:, :].rearrange("p (n t) -> p n t", t=TOK)

    def vtile(m):                     # v token-tile m: [128, 768]
        return vbuf[:, m * H:(m + 1) * H]

    def fftile(kt):                   # ff1 feature k-tile: [128, 1024]
        if kt < 12:
            return big[:, kt, :]
        if kt < 18:
            return ctxTv[:, kt - 12, :]
        return vbuf[:, (kt - 18) * TOK:(kt - 17) * TOK]

    P = Prog()
    CD = ("pe", "act", "dve", "dma")

    DMA_FULL_SYNC = os.environ.get("KERNEL_DMA_FULL_SYNC", "") != ""

    def dma(dst, src, deps=("pe", "act", "dve")):
        if DMA_FULL_SYNC:
            deps = CD
        P.emit("dma", lambda d=dst, s=src: nc.sync.dma_start(out=d, in_=s),
               deps=deps)

    def mm(out, lhsT, rhs, start, stop, raw=False):
        if raw:
            P.emit("pe", lambda o=out, l=lhsT, r=rhs, a=start, b=stop:
                   nc.tensor.matmul(o, l, r, start=a, stop=b), deps=CD)
        else:
            P.emit("pe", lambda o=out, l=lhsT, r=rhs, a=start, b=stop:
                   nc.tensor.matmul(o, mmdt(l), mmdt(r), start=a, stop=b),
                   deps=CD)

    def act(out, in_, func, b=0.0, scale=1.0):
        P.emit("act", lambda o=out, i=in_, f=func, bb=b, s=scale:
               nc.scalar.activation(o, i, f, bias=bb, scale=s), deps=CD)

    def dve_tt(out, in0, in1, op):
        P.emit("dve", lambda o=out, x=in0, y=in1, z=op:
               nc.vector.tensor_tensor(o, x, y, z), deps=CD)

    def dve_ts(out, in_, s1, s2, op0, op1):
        P.emit("dve", lambda o=out, i=in_, a=s1, b=s2, x=op0, y=op1:
               nc.vector.tensor_scalar(o, i, a, b, x, y), deps=CD)

    def dve_recip(out, in_):
        P.emit("dve", lambda o=out, i=in_: nc.vector.reciprocal(o, i), deps=CD)

    # ---- boot: constants + initial activations
    dma(csts[:, :], consts[:, 0:904], deps=())
    dma(R(onesr[:, :]), R(onesd[:, :]), deps=())
    dma(R(hT[:, :, :]), R(hT0.rearrange("(n p) t -> p n t", p=128)), deps=())
    ones128 = onesr[:, 0:1]
    onesrow = csts[0:1, 2:130]        # [1,128] ones on partition 0

    def m12(m):                       # [12, 128] head-broadcast map k-tile
        return csts[0:12, 130 + m * 128:130 + (m + 1) * 128]

    def stream_gemm(W_dram, n_in, n_out, rhs_tile_fn, out_fn, bias_fn,
                    act_fn):
        """out[m] = act(sum_kt W[kt,m].T @ rhs[kt] + bias[m]); W streamed
        through wsl slots (per m-tile)."""
        kt_n = n_in // 128
        mt_n = n_out // 128
        for m in range(mt_n):
            slot = wsl[:, m % 2, :]
            for kt in range(kt_n):
                dma(R(slot[:, kt * 128:(kt + 1) * 128]),
                    R(W_dram[kt * 128:(kt + 1) * 128, m * 128:(m + 1) * 128]),
                    deps=("pe",))
            for half in range(2):
                ps = psA[:, half * 512:(half + 1) * 512]
                for kt in range(kt_n):
                    mm(ps, slot[:, kt * 128:(kt + 1) * 128],
                       rhs_tile_fn(kt)[:, half * 512:(half + 1) * 512],
                       start=(kt == 0), stop=(kt == kt_n - 1))
            act(R(out_fn(m)), psA[:, :TOK], act_fn, b=bias_fn(m))

    def layernorm(x, gcol0, bcol0, gbuf, sq):
        # x: [128, KT, TOK] feature-major; returns in place
        for kt in range(KT):
            act(R(sq[:, kt, :]), x[:, kt, :], AF.Square)
        for half in range(2):
            c0, c1 = half * 512, (half + 1) * 512
            for kt in range(KT):
                mm(psA[0:1, c0:c1], ones128, x[:, kt, c0:c1],
                   start=(kt == 0), stop=(kt == KT - 1))
            for kt in range(KT):
                mm(psB[0:1, c0:c1], ones128, sq[:, kt, c0:c1],
                   start=(kt == 0), stop=(kt == KT - 1))
        mean = stats[0:1, 0:1024]
        blk = stats[0:1, 1024:2048]       # E2 -> var -> istd, in place
        tmp = stats2[0:1, :]              # meansq -> sd
        act(mean, psA[0:1, :], AF.Identity, scale=1.0 / H)
        act(blk, psB[0:1, :], AF.Identity, scale=1.0 / H)
        dve_tt(tmp, mean, mean, ALU.mult)
        dve_tt(blk, blk, tmp, ALU.subtract)
        P.emit("dve", lambda: nc.vector.tensor_scalar_add(blk, blk, 1e-12),
               deps=CD)
        act(tmp, blk, AF.Sqrt)
        dve_recip(blk, tmp)                                           # istd
        for half in range(2):
            c0, c1 = half * 512, (half + 1) * 512
            mm(psA[:, c0:c1], onesrow, stats[0:1, c0:c1], start=True,
               stop=True, raw=True)
            mm(psB[:, c0:c1], onesrow, stats[0:1, 1024 + c0:1024 + c1],
               start=True, stop=True, raw=True)
        for kt in range(KT):
            dve_tt(R(x[:, kt, :]), x[:, kt, :], psA[:, :TOK], ALU.subtract)
            dve_tt(R(x[:, kt, :]), x[:, kt, :], psB[:, :TOK], ALU.mult)
            dve_ts(R(x[:, kt, :]), x[:, kt, :],
                   gbuf[:, gcol0 + kt:gcol0 + kt + 1],
                   gbuf[:, bcol0 + kt:bcol0 + kt + 1], ALU.mult, ALU.add)

    for l in range(n_layers):
        P.next_dma_sem(f"dmaS{l + 1}")
        dma(bias[:, :], biasall[l])

        # qkT into big[:, 0:12]: features q(0-5) k(6-11)
        stream_gemm(Wqkv[l][:, 0:1536], H, 1536, lambda kt: hT[:, kt, :],
                    lambda m: big[:, m, :], lambda m: bias[:, m:m + 1],
                    AF.Identity)

        # v = hT.T @ Wv  (token-major; bias folded in after softmax)
        for kt in range(KT):
            dma(R(wsl[:, kt % 2, (kt // 2) * 768:(kt // 2) * 768 + 768]),
                R(Wqkv[l][kt * 128:(kt + 1) * 128, 1536:2304]), deps=("pe",))
        for m in range(MT_TOK):
            for c0, c1 in ((0, 512), (512, 768)):
                ps = psA[:, c0:c1]
                for kt in range(KT):
                    wv = wsl[:, kt % 2, (kt // 2) * 768:(kt // 2) * 768 + 768]
                    mm(ps, hT[:, kt, m * 128:(m + 1) * 128], wv[:, c0:c1],
                       start=(kt == 0), stop=(kt == KT - 1))
            act(R(vtile(m)), psA[:, 0:H], AF.Identity)

        # attention
        for s in range(BL):
            for hh in range(NH):
                prow = 64 * (hh % 2)
                qt = big[prow:prow + 64, hh // 2, s * S:(s + 1) * S]
                ktap = big[prow:prow + 64, 6 + hh // 2, s * S:(s + 1) * S]
                for i in range(2):
                    mm(psS[:, i, :], ktap[:, i * 128:(i + 1) * 128], qt,
                       start=True, stop=True)
                act(R(expS[:, :, :]), psS[:, :, :], AF.Exp, scale=1.0 / 8.0)
                for i in range(2):
                    mm(psC[0:64, :], vtile(2 * s + i)[:, hh * 64:(hh + 1) * 64],
                       expS[:, i, :], start=(i == 0), stop=(i == 1))
                    mm(psS[0:1, 0, :], ones128, expS[:, i, :],
                       start=(i == 0), stop=(i == 1))
                act(R(ctxTv[prow:prow + 64, hh // 2, s * S:(s + 1) * S]),
                    psC[0:64, :], AF.Identity)
                zr = zbuf4[32 * (hh % 4):32 * (hh % 4) + 1,
                           (hh // 4) * 1024 + s * S:(hh // 4) * 1024 + (s + 1) * S]
                act(zr, psS[0:1, 0, :], AF.Identity)

        # normalize ctx by Z (per head), add v bias
        for p4 in range(4):
            dve_recip(zbuf4[32 * p4:32 * p4 + 1, :], zbuf4[32 * p4:32 * p4 + 1, :])
        dma(zscr[:, :], zbuf4[0:97:32, :])
        # must wait for the zscr store above: DMAs from one queue are split
        # across 16 SDMA engines with no cross-DMA completion ordering
        dma(zbuf[0:12, :], zscr[:, :].rearrange("p (b t) -> (p b) t", b=3),
            deps=CD)
        for m in range(KT):
            for half in range(2):
                mm(psA[:, half * 512:(half + 1) * 512], m12(m),
                   zbuf[0:12, half * 512:(half + 1) * 512], start=True,
                   stop=True, raw=True)
            dve_tt(R(ctxTv[:, m, :]), ctxTv[:, m, :], psA[:, :TOK], ALU.mult)
            P.emit("dve", lambda m=m: nc.vector.tensor_scalar_add(
                R(ctxTv[:, m, :]), ctxTv[:, m, :], bias[:, 12 + m:13 + m]),
                deps=CD)

        # attn proj + residual + LN1
        stream_gemm(Wo[l], H, H, lambda kt: ctxTv[:, kt, :],
                    lambda m: h1T[:, m, :], lambda m: bias[:, 18 + m:19 + m],
                    AF.Identity)
        for m in range(KT):
            dve_tt(R(h1T[:, m, :]), h1T[:, m, :], hT[:, m, :], ALU.add)
        layernorm(h1T, 24, 30, bias, big[:, 0:KT, :])

        # FF1 (gelu) into big/ctxT/vbuf tiles
        stream_gemm(W1[l], H, FF, lambda kt: h1T[:, kt, :],
                    fftile, lambda m: bias[:, 36 + m:37 + m], AF.Gelu)

        # FF2 + residual + LN2 -> hT
        for m in range(KT):
            slot = wsl[:, m % 2, :]
            for kt in range(24):
                dma(R(slot[:, kt * 128:(kt + 1) * 128]),
                    R(W2[l][kt * 128:(kt + 1) * 128, m * 128:(m + 1) * 128]),
                    deps=("pe",))
            for half in range(2):
                ps = psA[:, half * 512:(half + 1) * 512]
                for kt in range(24):
                    mm(ps, slot[:, kt * 128:(kt + 1) * 128],
                       fftile(kt)[:, half * 512:(half + 1) * 512],
                       start=(kt == 0), stop=(kt == 23))
            act(R(hT[:, m, :]), psA[:, :TOK], AF.Identity, b=bias[:, 60 + m:61 + m])
            dve_tt(R(hT[:, m, :]), hT[:, m, :], h1T[:, m, :], ALU.add)
        layernorm(hT, 66, 72, bias, big[:, 0:KT, :])

    dma(hTout.rearrange("(n p) t -> p n t", p=128), hT[:, :, :])

    # ---- replay into engine blocks
    sems = {}
    for name in sorted(P.sem_names):
        sems[name] = ctx.enter_context(nc.semaphore(name))
    with nc.Block() as block:
        @block.tensor
        def _(eng):
            P.replay("pe", eng, sems)

        @block.scalar
        def _(eng):
            P.replay("act", eng, sems)

        @block.vector
        def _(eng):
            P.replay("dve", eng, sems)

        @block.sync
        def _(eng):
            P.replay("dma", eng, sems)

    return nc, ctx


def _pack_consts():
    c = np.zeros((128, 1024), np.float32)
    c[:, 0] = 1.0                       # ones128
    c[0, 2:130] = 1.0                   # onesrow
    # zbuf row r (after the strided reshape DMA) holds head (r%3)*4 + r//3
    for r in range(NH):
        hh = (r % 3) * 4 + r // 3
        for f in range(H):
            if f // DH == hh:
                c[r, 130 + f] = 1.0
    return c


def _pack_bias(a):
    out = np.zeros((NL, 128, 80), np.float32)

    def col(vec):                       # feature vec [n*128] -> [128, n]
        return vec.reshape(-1, 128).T

    for l in range(NL):
        out[l, :, 0:18] = col(a['bqkv'][l])
        out[l, :, 18:24] = col(a['bo'][l])
        out[l, :, 24:30] = col(a['ln1_g'][l])
        out[l, :, 30:36] = col(a['ln1_b'][l])
        out[l, :, 36:60] = col(a['b1'][l])
        out[l, :, 60:66] = col(a['b2'][l])
        out[l, :, 66:72] = col(a['ln2_g'][l])
        out[l, :, 72:78] = col(a['ln2_b'][l])
    return out


def run_device(h0, a):
    global LAST_HW_NS
    if not USE_DEVICE:
        return _bert_host(h0, a)
    import time
    from concourse.bass_utils import run_bass_kernel_spmd

    key = ("enc", DEV_LAYERS)
    if key not in _CACHE:
        _CACHE[key] = _build_encoder(DEV_LAYERS)
    nc, _ctx = _CACHE[key]

    biasall = _pack_bias(a)
    consts = _pack_consts()
    shared = {"Wqkv": a['Wqkv'], "Wo": a['Wo'], "W1": a['W1'], "W2": a['W2'],
              "biasall": biasall, "consts": consts,
              "onesd": np.ones((128, 1), np.float32)}
    in_maps = []
    for c in range(NCORES):
        hc = h0[c * BL:(c + 1) * BL].reshape(TOK, H).T.copy()  # [H, TOK]
        in_maps.append(dict(shared, hT0=np.ascontiguousarray(hc)))

    t0 = time.time()
    res = run_bass_kernel_spmd(nc, in_maps, list(range(NCORES)))
    LAST_HW_NS = int((time.time() - t0) * 1e9)
    if getattr(res, "exec_time_ns", None):
        LAST_HW_NS = int(res.exec_time_ns)
        try:
            print("[kernel] profile exec_time_ns:", res.exec_time_ns,
                  "trace:", res.instructions_and_trace and res.instructions_and_trace[1])
        except Exception:
            pass

    h = np.zeros((B, S, H), np.float32)
    for c in range(NCORES):
        h[c * BL:(c + 1) * BL] = res.results[c]["hTout"].T.reshape(BL, S, H)
    if DEV_LAYERS < NL:                 # debugging path: finish on host
        a2 = {k: (v[DEV_LAYERS:] if k in ("Wqkv", "bqkv", "Wo", "bo", "ln1_g",
              "ln1_b", "W1", "b1", "W2", "b2", "ln2_g", "ln2_b") else v)
              for k, v in a.items()}
        h = _bert_host(h, a2, NL - DEV_LAYERS)
    return h


def kernel(input_ids, attention_mask, labels, emb_tok, emb_pos, emb_type,
           ln_emb_g, ln_emb_b, Wqkv, bqkv, Wo, bo, ln1_g, ln1_b, W1, b1,
           W2, b2, ln2_g, ln2_b, Wih_f, Whh_f, bih_f, bhh_f, Wih_b, Whh_b,
           bih_b, bhh_b, Wc, bc, tag_weight, crf_start, crf_end, crf_trans):
    args = {k: np.asarray(v) for k, v in locals().items()}
    maskf = args['attention_mask'].astype(np.float32)

    h0 = (args['emb_tok'][args['input_ids']] + args['emb_pos'][:S][None]
          + args['emb_type'][0][None, None]).astype(np.float32)
    h0 = _ln_np(h0, args['ln_emb_g'], args['ln_emb_b'])

    h = run_device(h0, args)

    hf = _lstm_host(h, args['Wih_f'], args['Whh_f'], args['bih_f'], args['bhh_f'], False)
    hb = _lstm_host(h, args['Wih_b'], args['Whh_b'], args['bih_b'], args['bhh_b'], True)
    logits = (np.concatenate([hf, hb], -1) @ args['Wc'] + args['bc']) * args['tag_weight']
    ll = _crf_host(logits, args['labels'], maskf, args['crf_start'],
                   args['crf_end'], args['crf_trans'])
    return np.float32(ll.mean())

